# revision 4
# baseline (speedup 1.0000x reference)
"""Cost-volume kernel for Trainium2 (Bass/Tile), SPMD over 8 NeuronCores.

out[b,c,d,h,w] = left[b,c,h,w] * right[b,c,h,w-d]  (0 where w < d), clipped.

Sharding: channels C=32 split 4-per-core (identical SPMD program, cores differ
only in input data). Each core computes its [4, 64, 160, 320] slab; the host
concatenates along C.

The kernel is HBM-write-bound (each core stores a 4*64*160*320 volume), so the
whole device pipeline runs in float16: inputs are cast to f16 on the host
(halves the load bytes), the DVE multiply produces f16, and the stores move
half the bytes of the f32 baseline. The host upcasts the gathered volume back
to f32. f16 roundoff on a product of two unit normals is ~1e-3 relative -- far
inside the 2e-2 gate.

Per-core layout: rows (c,h) on the 128 SBUF partitions.
  - tiles 0..3: channel c, h in [0,128)          -> [128, W]
  - tile  4   : all 4 channels, h in [128,160)   -> [4*32, W] packed
The disparity shift is along W only, so rows are independent.

Compute: r is staged with a 64-column zero head (rpad[:, 0:64] = 0,
rpad[:, 64:384] = r), and the disparity axis is REVERSED (e = 63-d) so all
access-pattern strides stay positive:
    blk[p, e', w] = l[p, w] * rpad[p, 1 + e0 + e' + w]
                  = l[p, w] * r[p, w - d]          (0 where w < d)
One DVE op per (tile, 16-disparity group) computes the full-width product
INCLUDING the masked zeros (l times the zero head), so the output needs no
memsets and no ragged stores. With 2-byte operands the tensor_tensor multiply
runs in the DVE 2x perf mode (~2.7 us per 16-disparity block vs ~3.6 us for
its store), keeping the vector engine off the critical path.

Stores: the per-core output is laid out h-major [C_LOC, H, E=64, W] with
e = 63-d. For one h row, a 16-disparity group is 16*320*2 = 10 KB contiguous
in DRAM, so each (tile, group) is ONE big DMA of fully-contiguous-per-row
streams (real HBM write bandwidth punishes short scattered runs; 10 KB runs
are comfortably past the knee). The tail tile is a single DMA with a 3-dim
DRAM access pattern covering all 4 channels. Store DMAs alternate between the
SP and Activation HWDGE queues so descriptor-generation overhead on one queue
hides under the other queue's transfer.

The host gather reverses e and transposes (c, h, d, w) -> (c, d, h, w).
"""

import os

import numpy as np

os.environ.setdefault("NEURON_RT_RESET_CORES", "1")

import concourse.bass as bass
import concourse.tile as tile
from concourse import bacc, mybir
from concourse.bass_utils import run_bass_kernel_spmd

B, C, H, W = 1, 32, 160, 320
D = 64
N_CORES = 8
C_LOC = C // N_CORES          # 4 channels per core
H_MAIN = 128                  # h rows on partitions for the per-channel main tiles
H_TAIL = H - H_MAIN           # 32
N_TILES = C_LOC + 1           # 4 main + 1 packed tail
RPAD0 = D                     # zero-head columns of the staged right tiles
E_SPLIT = 4                   # disparity groups per tile (pipeline granularity)
EH = D // E_SPLIT             # 16 disparities per group

_cache = {}


def _build_program():
    nc = bacc.Bacc(
        "TRN2",
        target_bir_lowering=False,
        debug=False,
        enable_asserts=False,
        num_devices=N_CORES,
    )
    f16 = mybir.dt.float16
    left_d = nc.dram_tensor("left", [C_LOC, H, W], f16, kind="ExternalInput").ap()
    right_d = nc.dram_tensor("right", [C_LOC, H, W], f16, kind="ExternalInput").ap()
    # h-major, e-reversed: out[c, h, e, w] = vol[c, d=63-e, h, w]
    out_d = nc.dram_tensor("out", [C_LOC, H, D, W], f16, kind="ExternalOutput").ap()

    lts = [
        nc.alloc_sbuf_tensor(f"lt{t}", [128, W], f16).ap() for t in range(N_TILES)
    ]
    rts = [
        nc.alloc_sbuf_tensor(f"rt{t}", [128, RPAD0 + W], f16).ap()
        for t in range(N_TILES)
    ]

    with tile.TileContext(nc) as tc:
        with tc.tile_pool(name="outp", bufs=8) as outp:
            # Zero heads: rpad[p, 64+k] = r[p, k], rpad[p, <64] = 0 implements
            # the w<d mask for free.
            for t in range(N_TILES):
                nc.vector.memset(rts[t][:, 0:RPAD0], 0.0)
            # Tile 0 loads on SP so the first multiply unblocks immediately;
            # the rest stream in on the Activation queue behind it.
            nc.sync.dma_start(out=lts[0][:, :], in_=left_d[0, 0:H_MAIN, :])
            nc.sync.dma_start(out=rts[0][:, RPAD0:], in_=right_d[0, 0:H_MAIN, :])
            for t in range(1, C_LOC):
                nc.scalar.dma_start(out=lts[t][:, :], in_=left_d[t, 0:H_MAIN, :])
                nc.scalar.dma_start(
                    out=rts[t][:, RPAD0:], in_=right_d[t, 0:H_MAIN, :]
                )
            nc.scalar.dma_start(
                out=lts[C_LOC][:, :], in_=left_d[0:C_LOC, H_MAIN:H, :]
            )
            nc.scalar.dma_start(
                out=rts[C_LOC][:, RPAD0:], in_=right_d[0:C_LOC, H_MAIN:H, :]
            )

            k = 0
            for t in range(N_TILES):
                for s in range(E_SPLIT):
                    e0 = s * EH
                    blk = outp.tile(
                        [128, EH, W], f16, name=f"blk_{t}_{s}", tag="blk"
                    )
                    bb = blk[:, :, :]
                    l_bc = lts[t][:, :].unsqueeze(1).broadcast_to([128, EH, W])
                    r_base = rts[t][:, :]
                    rpitch = r_base.ap[0][0]
                    r_win = bass.AP(
                        r_base.tensor,
                        r_base.offset + 1 + e0,
                        [[rpitch, 128], [1, EH], [1, W]],
                    )
                    nc.vector.tensor_mul(bb, l_bc, r_win)

                    eng = nc.sync if k % 2 == 0 else nc.scalar
                    k += 1
                    if t < C_LOC:
                        eng.dma_start(out=out_d[t, 0:H_MAIN, e0 : e0 + EH, :], in_=bb)
                    else:
                        eng.dma_start(
                            out=out_d[0:C_LOC, H_MAIN:H, e0 : e0 + EH, :], in_=bb
                        )

    nc.compile()
    return nc


def kernel(**inputs):
    left = np.asarray(inputs["left"], dtype=np.float32)
    right = np.asarray(inputs["right"], dtype=np.float32)
    nd = int(np.asarray(inputs["num_disparities"]))
    assert left.shape == (B, C, H, W) and right.shape == (B, C, H, W)
    assert nd == D, f"kernel hardcodes num_disparities={D}, got {nd}"

    if "nc" not in _cache:
        _cache["nc"] = _build_program()
    nc = _cache["nc"]

    left16 = np.ascontiguousarray(left[0].astype(np.float16))
    right16 = np.ascontiguousarray(right[0].astype(np.float16))
    in_maps = [
        {
            "left": left16[i * C_LOC : (i + 1) * C_LOC],
            "right": right16[i * C_LOC : (i + 1) * C_LOC],
        }
        for i in range(N_CORES)
    ]
    res = run_bass_kernel_spmd(nc, in_maps, list(range(N_CORES)))
    _cache["last_results"] = res

    # per-core [C_LOC, H, E, W] f16 (e = 63-d) -> (C, D, H, W) f32
    full = np.concatenate([np.asarray(r["out"]) for r in res.results], axis=0)
    full = full[:, :, ::-1, :].transpose(0, 2, 1, 3).astype(np.float32)
    np.clip(full, -1000.0, 1000.0, out=full)
    return np.ascontiguousarray(full)[None]  # (1, 32, 64, 160, 320) float32


# revision 13
# speedup vs baseline: 1.0926x; 1.0926x over previous
"""Cost-volume kernel for Trainium2 (Bass/Tile), SPMD over 8 NeuronCores.

out[b,c,d,h,w] = left[b,c,h,w] * right[b,c,h,w-d]  (0 where w < d), clipped.

Sharding: channels C=32 split 4-per-core (identical SPMD program, cores differ
only in input data). Each core computes its [4, 64, 160, 320] slab; the host
concatenates along C.

The kernel is HBM-write-bound (each core stores a 4*64*160*320 volume), so the
whole device pipeline runs in float16: inputs are cast to f16 on the host
(halves the load bytes), the DVE multiply produces f16, and the stores move
half the bytes of the f32 baseline. The host upcasts the gathered volume back
to f32. f16 roundoff on a product of two unit normals is ~1e-3 relative -- far
inside the 2e-2 gate.

Per-core layout: rows (c,h) on the 128 SBUF partitions.
  - tiles 0..3: channel c, h in [0,128)          -> [128, W]
  - tile  4   : all 4 channels, h in [128,160)   -> [4*32, W] packed
The disparity shift is along W only, so rows are independent.

Compute: r is staged with a 64-column zero head (rpad[:, 0:64] = 0,
rpad[:, 64:384] = r), and the disparity axis is REVERSED (e = 63-d) so all
access-pattern strides stay positive:
    blk[p, e', w] = l[p, w] * rpad[p, 1 + e0 + e' + w]
                  = l[p, w] * r[p, w - d]          (0 where w < d)
One DVE op per (tile, 16-disparity group) computes the full-width product
INCLUDING the masked zeros (l times the zero head), so the output needs no
memsets and no ragged stores. With 2-byte operands the tensor_tensor multiply
runs in the DVE 2x perf mode (~2.7 us per 16-disparity block vs ~3.6 us for
its store), keeping the vector engine off the critical path.

Stores: the per-core output is laid out h-major [C_LOC, H, E=64, W] with
e = 63-d. For one h row, a 16-disparity group is 16*320*2 = 10 KB contiguous
in DRAM, so each (tile, group) is ONE big DMA of fully-contiguous-per-row
streams (real HBM write bandwidth punishes short scattered runs; 10 KB runs
are comfortably past the knee). The tail tile is a single DMA with a 3-dim
DRAM access pattern covering all 4 channels. Store DMAs alternate between the
SP and Activation HWDGE queues so descriptor-generation overhead on one queue
hides under the other queue's transfer.

The host gather reverses e and transposes (c, h, d, w) -> (c, d, h, w).
"""

import os

import numpy as np

os.environ.setdefault("NEURON_RT_RESET_CORES", "1")

import concourse.bass as bass
import concourse.tile as tile
from concourse import bacc, mybir
from concourse.bass_utils import run_bass_kernel_spmd

B, C, H, W = 1, 32, 160, 320
D = 64
N_CORES = 8
C_LOC = C // N_CORES          # 4 channels per core
H_MAIN = 128                  # h rows on partitions for the per-channel main tiles
H_TAIL = H - H_MAIN           # 32
N_TILES = C_LOC + 1           # 4 main + 1 packed tail
RPAD0 = D                     # zero-head columns of the staged right tiles
E_SPLIT = 8                   # disparity groups per tile (pipeline granularity)
EH = D // E_SPLIT             # 16 disparities per group

_cache = {}


def _build_program():
    nc = bacc.Bacc(
        "TRN2",
        target_bir_lowering=False,
        debug=False,
        enable_asserts=False,
        num_devices=N_CORES,
    )
    f16 = mybir.dt.float16
    left_d = nc.dram_tensor("left", [C_LOC, H, W], f16, kind="ExternalInput").ap()
    right_d = nc.dram_tensor("right", [C_LOC, H, W], f16, kind="ExternalInput").ap()
    # h-major, e-reversed: out[c, h, e, w] = vol[c, d=63-e, h, w] for h<128.
    # The packed tail rows live in their own partition-major tensor
    # out_t[p=(c*32+h'), e, w] so tail stores keep 3-dim access patterns even
    # with the zero-prefix skip below.
    out_d = nc.dram_tensor(
        "out", [C_LOC, H_MAIN, D, W], f16, kind="ExternalOutput"
    ).ap()
    out_t = nc.dram_tensor("out_t", [128, D, W], f16, kind="ExternalOutput").ap()

    # All 5 l/r tiles live side-by-side in two wide SBUF tensors so the bulk
    # of the input can stream in as ONE strided DMA per tensor (HWDGE
    # descriptor-generation is ~625ns per DMA instruction; ten small loads
    # would serialize there and leave the DMA engines idling during ramp-up).
    RT_P = RPAD0 + W              # 384-column pitch of one staged right tile
    l_all = nc.alloc_sbuf_tensor("l_all", [128, N_TILES * W], f16).ap()
    r_all = nc.alloc_sbuf_tensor("r_all", [128, N_TILES * RT_P], f16).ap()
    lts = [l_all[:, t * W : (t + 1) * W] for t in range(N_TILES)]
    rts = [r_all[:, t * RT_P : (t + 1) * RT_P] for t in range(N_TILES)]

    with tile.TileContext(nc) as tc:
        with tc.tile_pool(name="outp", bufs=8) as outp:
            # Zero heads: rpad[p, 64+k] = r[p, k], rpad[p, <64] = 0 implements
            # the w<d mask for free. One strided memset covers all 5 heads.
            zpitch = r_all.ap[0][0]
            zhead = bass.AP(
                r_all.tensor,
                r_all.offset,
                [[zpitch, 128], [RT_P, N_TILES], [1, RPAD0]],
            )
            nc.vector.memset(zhead, 0.0)
            # Tile 0 loads first so the first multiply unblocks immediately:
            # l0 on the SP HWDGE queue, r0 through the gpsimd SWDGE path whose
            # descriptor generation runs on the otherwise-idle Pool engine in
            # parallel with l0's HWDGE pass. Tiles 1-3 follow as one strided
            # DMA per input; the packed tail rides on the SWDGE path behind r0.
            nc.sync.dma_start(out=lts[0][:, :], in_=left_d[0, 0:H_MAIN, :])
            nc.gpsimd.dma_start(out=rts[0][:, RPAD0:], in_=right_d[0, 0:H_MAIN, :])
            lpitch = l_all.ap[0][0]
            l_rest = bass.AP(
                l_all.tensor,
                l_all.offset + W,
                [[lpitch, 128], [W, C_LOC - 1], [1, W]],
            )
            r_rest = bass.AP(
                r_all.tensor,
                r_all.offset + RT_P + RPAD0,
                [[zpitch, 128], [RT_P, C_LOC - 1], [1, W]],
            )
            ld_main = bass.AP(
                left_d.tensor, left_d.offset + H * W, [[W, H_MAIN], [H * W, C_LOC - 1], [1, W]]
            )
            rd_main = bass.AP(
                right_d.tensor, right_d.offset + H * W, [[W, H_MAIN], [H * W, C_LOC - 1], [1, W]]
            )
            nc.sync.dma_start(out=l_rest, in_=ld_main)
            nc.scalar.dma_start(out=r_rest, in_=rd_main)
            nc.gpsimd.dma_start(
                out=lts[C_LOC][:, :], in_=left_d[0:C_LOC, H_MAIN:H, :]
            )
            nc.gpsimd.dma_start(
                out=rts[C_LOC][:, RPAD0:], in_=right_d[0:C_LOC, H_MAIN:H, :]
            )

            # Disparity groups per tile. Tile 0's first group is split in two
            # so the very first store issues ~1.3us earlier (the ramp is the
            # only DMA idle time besides the fixed drain).
            groups = [(s * EH, EH) for s in range(E_SPLIT)]
            groups0 = [(0, EH // 2), (EH // 2, EH // 2)] + groups[1:]

            k = 0
            for t in range(N_TILES):
                for e0, eh in groups0 if t == 0 else groups:
                    blk = outp.tile(
                        [128, EH, W], f16, name=f"blk_{t}_{e0}", tag="blk"
                    )
                    bb = blk[:, 0:eh, :]
                    l_bc = lts[t][:, :].unsqueeze(1).broadcast_to([128, eh, W])
                    r_base = rts[t][:, :]
                    rpitch = r_base.ap[0][0]
                    r_win = bass.AP(
                        r_base.tensor,
                        r_base.offset + 1 + e0,
                        [[rpitch, 128], [1, eh], [1, W]],
                    )
                    nc.vector.tensor_mul(bb, l_bc, r_win)

                    eng = nc.sync if k % 2 == 0 else nc.scalar
                    k += 1
                    # Every disparity in this group has d >= dmin, so columns
                    # w < dmin are zero for the whole group. The PJRT runtime
                    # hands the kernel pre-zeroed output buffers (bass2jax
                    # donates zeroed arrays; the native runner pre-zeros too),
                    # so those columns need no store at all.
                    dmin = D - e0 - eh
                    if t < C_LOC:
                        eng.dma_start(
                            out=out_d[t, :, e0 : e0 + eh, dmin:],
                            in_=blk[:, 0:eh, dmin:],
                        )
                    else:
                        eng.dma_start(
                            out=out_t[:, e0 : e0 + eh, dmin:],
                            in_=blk[:, 0:eh, dmin:],
                        )

    nc.compile()
    return nc


def kernel(**inputs):
    left = np.asarray(inputs["left"], dtype=np.float32)
    right = np.asarray(inputs["right"], dtype=np.float32)
    nd = int(np.asarray(inputs["num_disparities"]))
    assert left.shape == (B, C, H, W) and right.shape == (B, C, H, W)
    assert nd == D, f"kernel hardcodes num_disparities={D}, got {nd}"

    if "nc" not in _cache:
        _cache["nc"] = _build_program()
    nc = _cache["nc"]

    left16 = np.ascontiguousarray(left[0].astype(np.float16))
    right16 = np.ascontiguousarray(right[0].astype(np.float16))
    in_maps = [
        {
            "left": left16[i * C_LOC : (i + 1) * C_LOC],
            "right": right16[i * C_LOC : (i + 1) * C_LOC],
        }
        for i in range(N_CORES)
    ]
    res = run_bass_kernel_spmd(nc, in_maps, list(range(N_CORES)))
    _cache["last_results"] = res

    # per-core out [C_LOC, 128, E, W] + out_t [128, E, W] f16 (e = 63-d)
    # -> (C, D, H, W) f32
    parts = []
    for r in res.results:
        main = np.asarray(r["out"])
        tail = np.asarray(r["out_t"]).reshape(C_LOC, H_TAIL, D, W)
        parts.append(np.concatenate([main, tail], axis=1))
    full = np.concatenate(parts, axis=0)
    full = full[:, :, ::-1, :].transpose(0, 2, 1, 3).astype(np.float32)
    np.clip(full, -1000.0, 1000.0, out=full)
    return np.ascontiguousarray(full)[None]  # (1, 32, 64, 160, 320) float32
